# revision 1
# baseline (speedup 1.0000x reference)
"""Transformer block (pre-norm attn + MLP) on 8 NeuronCores, data-parallel over batch.

Full inputs in, full outputs out. Each core processes one batch element
x[i] : [1024, 768] through an identical Bass/Tile kernel.

Host-side exact refactoring:
  - LN gains fold into the following matmul weights: diag(g) @ W.
  - LN biases fold into: per-column bias on q/k (applied during psum->sbuf
    copy), b_proj_eff = b_proj + (b1 @ w_qkv_v) @ w_proj (softmax rows sum
    to one, so a v-bias passes through attention additively), and
    b_fc1_eff = b_fc1 + b2 @ w_fc1.
  - w_proj rows are re-laid-out head-aligned: block h occupies rows
    h*128+1 .. h*128+97 (row 0 of each block pairs with the attention
    colsum row; zero), matching the on-chip o layout.
  - Weights are cast to bf16 on host. All matmuls run bf16 x bf16 with
    fp32 PSUM accumulation; the residual stream, layernorm statistics and
    softmax normalization stay fp32.

On-chip dataflow (per core):
  LN1 (token-major, bn_stats, fp32 in -> bf16 out) -> XBAR DMA-transpose
    -> h_fm [C, N] bf16
  qkv: q_fm/k_fm per-head feature-major [128pad, H, N]; v token-major with a
       leading ones column per head -> v_ext [N, H, 1+96]
  attn per (head, half): S^T = k.T q (psum fp32) -> exp on ACT -> E bf16;
       PV: o_unnorm[(1+96), n] = v_ext.T @ E  (row 0 = colsum);
       rinv = 1/colsum, gpsimd partition-broadcast, o = o_unnorm * rinv
  proj: x1 = x + o @ w_proj_p + b_proj_eff (fp32)
  LN2 -> DMA-transpose -> h2_fm bf16; MLP streamed over ff tiles:
       g = gelu(w_fc1.T h2 + b_fc1_eff) bf16; x2 += g.T w_fc2; + x1 + b_fc2
"""
import numpy as np
import ml_dtypes

import concourse.bass as bass
from concourse import bacc, mybir
from concourse.bass_utils import run_bass_kernel_spmd
from concourse.masks import make_identity
from concourse.tile import TileContext

P = 128
N = 1024          # tokens per core (batch element)
C = 768           # model dim
H = 8             # heads
DH = C // H       # 96
DFF = 4 * C       # 3072
NT = N // P       # 8 token tiles
KT = C // P       # 6 feature tiles
FFT = DFF // P    # 24 ff tiles
NH = 2            # halves of the token axis for attention
NC_ = N // NH     # 512
EPS = 1e-5
SCALE = DH ** -0.5
VW = DH           # per-head v width (plus a leading ones column)

F32 = mybir.dt.float32
BF16 = mybir.dt.bfloat16

_CACHED = {}


def build(taps=()):
    nc = bacc.Bacc("TRN2", debug=False)

    x_d = nc.dram_tensor("x", [N, C], F32, kind="ExternalInput")
    wqkv_d = nc.dram_tensor("w_qkv_e", [C, 3 * C], BF16, kind="ExternalInput")
    wproj_d = nc.dram_tensor("w_proj_p", [H * P, C], BF16, kind="ExternalInput")
    wfc1_d = nc.dram_tensor("w_fc1_e", [C, DFF], BF16, kind="ExternalInput")
    wfc2_d = nc.dram_tensor("w_fc2", [DFF, C], BF16, kind="ExternalInput")
    qkb_d = nc.dram_tensor("qk_bias", [P, 2 * H], F32, kind="ExternalInput")
    bp_d = nc.dram_tensor("b_proj_e", [C], F32, kind="ExternalInput")
    bf1_d = nc.dram_tensor("b_fc1_e", [DFF], F32, kind="ExternalInput")
    bf2_d = nc.dram_tensor("b_fc2", [C], F32, kind="ExternalInput")
    y_d = nc.dram_tensor("y", [N, C], F32, kind="ExternalOutput")

    tap_d = {}
    for name, shape, dt in [
        ("h_fm", [C, N], BF16),
        ("q_fm", [H * P, N], BF16),
        ("k_fm", [H * P, N], BF16),
        ("v_ext", [N, H * (VW + 1)], BF16),
        ("o_fm", [H * P, N], BF16),
        ("x1", [N, C], F32),
        ("h2_fm", [C, N], BF16),
    ]:
        if name in taps:
            tap_d[name] = nc.dram_tensor(
                "tap_" + name, shape, dt, kind="ExternalOutput"
            )

    def bcast_row(dram_t, width):
        return bass.AP(tensor=dram_t, offset=0, ap=[[0, P], [1, width]])

    with TileContext(nc) as tc:
        consts = tc.alloc_tile_pool(name="consts", bufs=1, side="left")
        pst = tc.alloc_tile_pool(name="pst", bufs=2, space="PSUM")

        # ---------- constants ----------
        ident = consts.tile([P, P], BF16)
        make_identity(nc, ident)
        eps_t = consts.tile([P, 1], F32)
        nc.vector.memset(eps_t, EPS)
        qkb = consts.tile([P, 2 * H], F32)
        nc.gpsimd.dma_start(qkb[:], qkb_d[:, :])
        bf1c = consts.tile([P, FFT], F32)
        nc.gpsimd.dma_start(bf1c[:], bf1_d.rearrange("(t p) -> p t", p=P))

        # ---------- helpers ----------
        def layernorm_tile(x_ap, h_tile, lnt, stats):
            """h_tile[:] = (x_ap - mean) * rstd, cast bf16."""
            st = stats.tile([P, 3, nc.vector.BN_STATS_DIM], F32, tag="bnst")
            for i in range(3):
                nc.vector.bn_stats(
                    out=st[:, i, :], in_=x_ap[:, i * 256:(i + 1) * 256]
                )
            mv = stats.tile([P, nc.vector.BN_AGGR_DIM], F32, tag="bnmv")
            nc.vector.bn_aggr(out=mv[:], in_=st[:])
            rstd = stats.tile([P, 1], F32, tag="rstd")
            nc.scalar.activation(
                out=rstd[:], in_=mv[:, 1:2],
                func=mybir.ActivationFunctionType.Sqrt,
                bias=eps_t[:], scale=1.0,
            )
            nc.vector.reciprocal(out=rstd[:], in_=rstd[:])
            nmr = stats.tile([P, 1], F32, tag="nmr")
            nc.vector.tensor_mul(nmr[:], mv[:, 0:1], rstd[:])
            nc.vector.tensor_scalar_mul(nmr[:], nmr[:], -1.0)
            nc.scalar.activation(
                out=h_tile[:], in_=x_ap[:],
                func=mybir.ActivationFunctionType.Identity,
                bias=nmr[:], scale=rstd[:],
            )

        def transpose_into(h_tile, dst_fm, nt):
            """dst_fm[:, kt, nt*P:(nt+1)*P] = h_tile[P, C].T via PE."""
            for kt in range(KT):
                tp = pst.tile([P, P], BF16, tag="tp")
                nc.tensor.transpose(
                    tp[:], h_tile[:, kt * P:(kt + 1) * P], ident[:]
                )
                nc.vector.tensor_copy(
                    dst_fm[:, kt, nt * P:(nt + 1) * P], tp[:]
                )

        # ---------- phase 1-2: LN1 + transpose ----------
        xpool = tc.alloc_tile_pool(name="xpool", bufs=1, side="left")
        hfmp = tc.alloc_tile_pool(name="hfmp", bufs=1, side="left")
        lnt1 = tc.alloc_tile_pool(name="lnt1", bufs=3, side="left")
        stats1 = tc.alloc_tile_pool(name="stats1", bufs=4, side="left")

        x_tok = xpool.tile([P, NT, C], F32)
        for nt in range(NT):
            nc.sync.dma_start(x_tok[:, nt, :], x_d[nt * P:(nt + 1) * P, :])
        h_fm = hfmp.tile([P, KT, N], BF16)
        for nt in range(NT):
            h_t = lnt1.tile([P, C], BF16, tag="h1")
            layernorm_tile(x_tok[:, nt, :], h_t, lnt1, stats1)
            transpose_into(h_t, h_fm, nt)

        if "h_fm" in tap_d:
            nc.sync.dma_start(
                tap_d["h_fm"].rearrange("(kt p) n -> p kt n", p=P), h_fm[:]
            )
        stats1.release()
        lnt1.release()

        # ---------- phase 3: qkv ----------
        # right-stack bottom: pools that live to the end (x1, wfc1)
        x1pool = tc.alloc_tile_pool(name="x1pool", bufs=1, side="right")
        wfc1p = tc.alloc_tile_pool(name="wfc1p", bufs=1, side="right")
        wqkvp = tc.alloc_tile_pool(name="wqkvp", bufs=1, side="left")
        qkvpool = tc.alloc_tile_pool(name="qkvpool", bufs=1, side="right")
        qkps = tc.alloc_tile_pool(name="qkps", bufs=3, space="PSUM")
        vps = tc.alloc_tile_pool(name="vps", bufs=3, space="PSUM")

        wqkv = wqkvp.tile([P, KT, 3 * C], BF16)
        for c0 in range(0, 3 * C, 512):
            cw = min(512, 3 * C - c0)
            nc.sync.dma_start(
                wqkv[:, :, c0:c0 + cw],
                wqkv_d.rearrange("(kt p) o -> p kt o", p=P)[:, :, c0:c0 + cw],
            )
        x1_tok = x1pool.tile([P, NT, C], F32)
        wfc1 = wfc1p.tile([P, KT, DFF], BF16)
        nc.sync.dma_start(wfc1[:], wfc1_d.rearrange("(kt p) f -> p kt f", p=P))

        q_fm = qkvpool.tile([P, H, N], BF16)
        k_fm = qkvpool.tile([P, H, N], BF16)
        for h in range(H):
            for which, dst in ((0, q_fm), (1, k_fm)):
                col0 = which * C + h * DH
                for nh in range(NH):
                    pq = qkps.tile([P, NC_], F32, tag="qk")
                    for kt in range(KT):
                        nc.tensor.matmul(
                            pq[:DH, :],
                            wqkv[:, kt, col0:col0 + DH],
                            h_fm[:, kt, nh * NC_:(nh + 1) * NC_],
                            start=(kt == 0), stop=(kt == KT - 1),
                        )
                    nc.vector.tensor_scalar_add(
                        dst[:DH, h, nh * NC_:(nh + 1) * NC_],
                        pq[:DH, :],
                        qkb[:DH, which * H + h:which * H + h + 1],
                    )

        # v generation: token-major, per-head slots [ones | v(96)]
        v_ext = qkvpool.tile([P, NT, H, VW + 1], BF16)
        for nt in range(NT):
            for half in range(2):  # 4 heads (384 cols) per psum
                pv = vps.tile([P, 4 * DH], F32, tag="v")
                c0 = 2 * C + half * 4 * DH
                for kt in range(KT):
                    nc.tensor.matmul(
                        pv[:],
                        h_fm[:, kt, nt * P:(nt + 1) * P],
                        wqkv[:, kt, c0:c0 + 4 * DH],
                        start=(kt == 0), stop=(kt == KT - 1),
                    )
                nc.vector.tensor_copy(
                    v_ext[:, nt, half * 4:(half + 1) * 4, 1:VW + 1],
                    pv.rearrange("p (h d) -> p h d", d=DH),
                )

        if "q_fm" in tap_d:
            nc.sync.dma_start(
                tap_d["q_fm"].rearrange("(h p) n -> p h n", p=P), q_fm[:]
            )
        if "k_fm" in tap_d:
            nc.sync.dma_start(
                tap_d["k_fm"].rearrange("(h p) n -> p h n", p=P), k_fm[:]
            )
        if "v_ext" in tap_d:
            nc.sync.dma_start(
                tap_d["v_ext"].rearrange("(nt p) (h w) -> p nt h w", p=P, w=VW + 1),
                v_ext[:],
            )

        vps.release()
        qkps.release()
        wqkvp.release()
        hfmp.release()

        # ---------- phase 4: attention per head ----------
        opool = tc.alloc_tile_pool(name="opool", bufs=1, side="left")
        wprojp = tc.alloc_tile_pool(name="wprojp", bufs=1, side="left")
        epool = tc.alloc_tile_pool(name="epool", bufs=2, side="right")
        rrow = tc.alloc_tile_pool(name="rrow", bufs=2, side="right")
        sps = tc.alloc_tile_pool(name="sps", bufs=2, space="PSUM")
        pvps = tc.alloc_tile_pool(name="pvps", bufs=2, space="PSUM")

        wproj = wprojp.tile([P, H, C], BF16)
        nc.sync.dma_start(wproj[:], wproj_d.rearrange("(hb p) c -> p hb c", p=P))
        bpb = wprojp.tile([P, C], F32)
        nc.gpsimd.dma_start(bpb[:], bcast_row(bp_d, C))
        o_fm = opool.tile([P, H, N], BF16)
        nc.gpsimd.memset(q_fm[DH:P, :, :], 0.0)
        nc.gpsimd.memset(k_fm[DH:P, :, :], 0.0)
        nc.gpsimd.memset(v_ext[:, :, :, 0], 1.0)
        nc.gpsimd.memset(o_fm[:], 0.0)
        pairs = [(h, nh) for h in range(H) for nh in range(NH)]

        def emit_scores(h, nh):
            e_t = epool.tile([P, NT, NC_], BF16, tag="E", name=f"e_{h}_{nh}")
            for mt2 in range(NT // 2):
                ps_s = sps.tile([P, 2, NC_], F32, tag="S",
                                name=f"s_{h}_{nh}_{mt2}")
                for sub in range(2):
                    nc.tensor.matmul(
                        ps_s[:, sub, :],
                        k_fm[:, h, (2 * mt2 + sub) * P:(2 * mt2 + sub + 1) * P],
                        q_fm[:, h, nh * NC_:(nh + 1) * NC_],
                        start=True, stop=True,
                    )
                nc.scalar.activation(
                    out=e_t[:, 2 * mt2:2 * mt2 + 2, :], in_=ps_s[:],
                    func=mybir.ActivationFunctionType.Exp,
                    bias=0.0, scale=SCALE,
                )
            return e_t

        def emit_pv(h, nh, e_t):
            po = pvps.tile([P, NC_], F32, tag="PV", name=f"po_{h}_{nh}")
            for mt in range(NT):
                nc.tensor.matmul(
                    po[:VW + 1, :],
                    v_ext[:, mt, h, :],
                    e_t[:, mt, :],
                    start=(mt == 0), stop=(mt == NT - 1),
                )
            rs = rrow.tile([1, NC_], F32, tag="rs", name=f"rs_{h}_{nh}")
            nc.vector.reciprocal(out=rs[0:1, :], in_=po[0:1, :])
            rb = rrow.tile([P, NC_], F32, tag="rb", name=f"rb_{h}_{nh}")
            nc.gpsimd.partition_broadcast(rb[:VW + 1, :], rs[0:1, :])
            nc.vector.tensor_mul(
                o_fm[0:VW + 1, h, nh * NC_:(nh + 1) * NC_],
                po[0:VW + 1, :], rb[0:VW + 1, :],
            )

        def emit_proj(nt):
            for c0, cw in ((0, 512), (512, 256)):
                pj = pjps.tile([P, 512], F32, tag="PJ", name=f"pj_{nt}_{c0}")
                for hb in range(H):
                    nc.tensor.matmul(
                        pj[:, :cw],
                        o_fm[:, hb, nt * P:(nt + 1) * P],
                        wproj[:, hb, c0:c0 + cw],
                        start=(hb == 0), stop=(hb == H - 1),
                    )
                nc.vector.tensor_add(
                    x1_tok[:, nt, c0:c0 + cw],
                    pj[:, :cw], x_tok[:, nt, c0:c0 + cw],
                )
                nc.vector.tensor_add(
                    x1_tok[:, nt, c0:c0 + cw],
                    x1_tok[:, nt, c0:c0 + cw], bpb[:, c0:c0 + cw],
                )

        pjps = None  # allocated after attention psum frees
        prev = None
        for (h, nh) in [(h, nh) for h in range(H) for nh in range(NH)]:
            e_t = emit_scores(h, nh)
            if prev is not None:
                emit_pv(*prev)
            prev = (h, nh, e_t)
        emit_pv(*prev)
        pvps.release()
        sps.release()
        pjps = tc.alloc_tile_pool(name="pjps", bufs=6, space="PSUM")
        for nt in range(NT):
            emit_proj(nt)
        pjps.release()

        if "o_fm" in tap_d:
            nc.sync.dma_start(
                tap_d["o_fm"].rearrange("(h p) n -> p h n", p=P), o_fm[:]
            )

        rrow.release()
        epool.release()
        qkvpool.release()

        # ---------- phase 5: proj tail emitted above ----------
        if "x1" in tap_d:
            nc.sync.dma_start(
                tap_d["x1"].rearrange("(nt p) c -> p nt c", p=P), x1_tok[:]
            )

        wprojp.release()
        opool.release()
        xpool.release()

        # ---------- phase 6-7: LN2 + transpose ----------
        h2p = tc.alloc_tile_pool(name="h2p", bufs=1, side="left")
        wfc2s = tc.alloc_tile_pool(name="wfc2s", bufs=6, side="right")
        lnt2 = tc.alloc_tile_pool(name="lnt2", bufs=3, side="left")
        stats2 = tc.alloc_tile_pool(name="stats2", bufs=4, side="left")

        h2_fm = h2p.tile([P, KT, N], BF16)
        for nt in range(NT):
            h_t = lnt2.tile([P, C], BF16, tag="h2")
            layernorm_tile(x1_tok[:, nt, :], h_t, lnt2, stats2)
            transpose_into(h_t, h2_fm, nt)

        if "h2_fm" in tap_d:
            nc.sync.dma_start(
                tap_d["h2_fm"].rearrange("(kt p) n -> p kt n", p=P), h2_fm[:]
            )

        stats2.release()
        lnt2.release()
        pst.release()

        # ---------- phase 8: MLP ----------
        mlpc = tc.alloc_tile_pool(name="mlpc", bufs=1, side="left")
        bf2b = mlpc.tile([P, C], F32)
        nc.gpsimd.dma_start(bf2b[:], bcast_row(bf2_d, C))
        gpool = tc.alloc_tile_pool(name="gpool", bufs=3, side="left")
        outs = tc.alloc_tile_pool(name="outs", bufs=2, side="left")
        gps = tc.alloc_tile_pool(name="gps", bufs=3, space="PSUM")
        x2ps = tc.alloc_tile_pool(name="x2ps", bufs=2, space="PSUM")

        QW = 256  # token quarter width
        for q in range(4):
            pa = [x2ps.tile([P, 512], F32, tag="x2a", name=f"pa{q}_{jj}")
                  for jj in range(2)]
            pb = [x2ps.tile([P, 256], F32, tag="x2b", name=f"pb{q}_{jj}")
                  for jj in range(2)]
            for ff in range(FFT):
                w2 = wfc2s.tile([P, C], BF16, tag="w2", name=f"w2_{q}_{ff}")
                nc.sync.dma_start(w2[:], wfc2_d[ff * P:(ff + 1) * P, :])
                pg = gps.tile([P, QW], F32, tag="G")
                for kt in range(KT):
                    nc.tensor.matmul(
                        pg[:],
                        wfc1[:, kt, ff * P:(ff + 1) * P],
                        h2_fm[:, kt, q * QW:(q + 1) * QW],
                        start=(kt == 0), stop=(kt == KT - 1),
                    )
                g_t = gpool.tile([P, QW], BF16, tag="g")
                nc.scalar.activation(
                    out=g_t[:], in_=pg[:],
                    func=mybir.ActivationFunctionType.Gelu,
                    bias=bf1c[:, ff:ff + 1], scale=1.0,
                )
                for j in range(2):
                    nc.tensor.matmul(
                        pa[j][:],
                        g_t[:, j * P:(j + 1) * P],
                        w2[:, 0:512],
                        start=(ff == 0), stop=(ff == FFT - 1),
                    )
                    nc.tensor.matmul(
                        pb[j][:],
                        g_t[:, j * P:(j + 1) * P],
                        w2[:, 512:768],
                        start=(ff == 0), stop=(ff == FFT - 1),
                    )
            for j in range(2):
                nt = 2 * q + j
                o_t = outs.tile([P, C], F32, tag="y")
                nc.vector.tensor_add(
                    o_t[:, 0:512], pa[j][:], x1_tok[:, nt, 0:512]
                )
                nc.vector.tensor_add(
                    o_t[:, 512:768], pb[j][:], x1_tok[:, nt, 512:768]
                )
                nc.vector.tensor_add(o_t[:], o_t[:], bf2b[:])
                nc.sync.dma_start(y_d[nt * P:(nt + 1) * P, :], o_t[:])

        x2ps.release()
        gps.release()
        outs.release()
        gpool.release()
        mlpc.release()
        wfc2s.release()
        h2p.release()
        wfc1p.release()
        x1pool.release()
        consts.release()

    nc.compile()
    return nc


def _prep_inputs(inputs):
    """Host-side prep (exact refactoring of LN gains/biases into weights)."""
    f = lambda k: np.asarray(inputs[k], dtype=np.float32)
    x = f("x")
    w_qkv, w_proj, w_fc1, w_fc2 = f("w_qkv"), f("w_proj"), f("w_fc1"), f("w_fc2")
    ln1_g, ln1_b, ln2_g, ln2_b = f("ln1_g"), f("ln1_b"), f("ln2_g"), f("ln2_b")
    b_proj, b_fc1, b_fc2 = f("b_proj"), f("b_fc1"), f("b_fc2")

    bf = ml_dtypes.bfloat16
    w_qkv_e = ln1_g[:, None] * w_qkv
    qkv_bias = ln1_b @ w_qkv  # [2304]
    qk_bias = np.zeros((P, 2 * H), dtype=np.float32)
    for which in range(2):
        for h in range(H):
            qk_bias[0:DH, which * H + h] = qkv_bias[
                which * C + h * DH: which * C + (h + 1) * DH
            ]
    vb = qkv_bias[2 * C: 3 * C]  # v bias passes through softmax additively
    b_proj_e = b_proj + vb @ w_proj
    # head-aligned w_proj rows: block h rows 1..97 (row 0 pairs with colsum row)
    w_proj_p = np.zeros((H * P, C), dtype=np.float32)
    for h in range(H):
        w_proj_p[h * P + 1: h * P + 1 + DH, :] = w_proj[h * DH:(h + 1) * DH, :]
    w_fc1_e = ln2_g[:, None] * w_fc1
    b_fc1_e = b_fc1 + ln2_b @ w_fc1

    common = {
        "w_qkv_e": np.ascontiguousarray(w_qkv_e.astype(bf)),
        "w_proj_p": np.ascontiguousarray(w_proj_p.astype(bf)),
        "w_fc1_e": np.ascontiguousarray(w_fc1_e.astype(bf)),
        "w_fc2": np.ascontiguousarray(w_fc2.astype(bf)),
        "qk_bias": qk_bias,
        "b_proj_e": b_proj_e,
        "b_fc1_e": b_fc1_e,
        "b_fc2": b_fc2,
    }
    return [dict(common, x=np.ascontiguousarray(x[i])) for i in range(8)]


def kernel(**inputs):
    if "nc" not in _CACHED:
        _CACHED["nc"] = build()
    nc = _CACHED["nc"]
    in_maps = _prep_inputs(inputs)
    res = run_bass_kernel_spmd(nc, in_maps, core_ids=list(range(8)))
    out = np.stack([res.results[i]["y"] for i in range(8)], axis=0)
    return out.astype(np.float32)



# revision 13
# speedup vs baseline: 1.1184x; 1.1184x over previous
"""Transformer block (pre-norm attn + MLP) on 8 NeuronCores, data-parallel over batch.

Full inputs in, full outputs out. Each core processes one batch element
x[i] : [1024, 768] through an identical Bass/Tile kernel.

Host-side exact refactoring (as baseline):
  - LN gains fold into following matmul weights: diag(g) @ W.
  - LN biases fold into q/k per-column bias, b_proj_eff, b_fc1_eff.
  - w_proj rows head-aligned: block h rows 1..97; block-0 row 0 carries
    b_proj_eff (o_fm row 0 is exactly 1.0 after normalization).
  - w_qkv / w_proj cast to fp8 e4m3; w_fc1 / w_fc2 to bf16.

v2.1 on-chip structure:
  - qkv / v-gen / PV / proj matmuls run fp8 DoubleRow (2 contraction
    tiles fused per instruction, 2x rate on HW). Scores stay bf16
    (K=96 gains nothing from DoubleRow).
  - merged qkv+attention software pipeline: v-gen first, then per-head
    steps qk(h) | S(h-1)+exp | PV(h-2)+norm so the exp stream overlaps
    the qkv matmuls.
  - exp split across engines (ACT direct exp; Schraudolph int32
    bit-trick on DVE + fp8 cast on DVE/GpSimd) with a -ln64 bias so
    e^S/64 <= ~66 stays well under the fp8 e4m3 max finite 240.
  - softmax normalization: ACT copies the colsum row out of PSUM,
    GpSimd broadcasts, DVE fast-reciprocal + multiply -> o_fm fp8.
  - wfc2 resident in SBUF; LN2 interleaved with the MLP per token
    quarter so fc matmuls start as soon as their h2 quarter is ready.
"""
import numpy as np
import ml_dtypes

import concourse.bass as bass
from concourse import bacc, mybir
from concourse.bass_utils import run_bass_kernel_spmd
from concourse.masks import make_identity
from concourse.tile import TileContext

P = 128
N = 1024          # tokens per core (batch element)
C = 768           # model dim
H = 8             # heads
DH = C // H       # 96
DFF = 4 * C       # 3072
NT = N // P       # 8 token tiles
KT = C // P       # 6 feature tiles
FFT = DFF // P    # 24 ff tiles
NH = 2            # halves of the token axis for attention
NC_ = N // NH     # 512
EPS = 1e-5
SCALE = DH ** -0.5
VW = DH           # per-head v width
VWE = VW + 2      # slot: [ones | v(96) | zero pad] (dual-fp8 ldweights needs even width)

F32 = mybir.dt.float32
BF16 = mybir.dt.bfloat16
F8 = mybir.dt.float8e4
I32 = mybir.dt.int32
DR = mybir.MatmulPerfMode.DoubleRow

# exp(x) ~= bitcast_f32(int32(x * 2^23*log2e + (127*2^23 - C0)))
LOG2E = 1.4426950408889634
SH_C = 486411.0           # min-RMS Schraudolph correction
EXP_A = (1 << 23) * LOG2E
EXP_B = 127 * (1 << 23) - SH_C
LNB = float(np.log(64.0))  # E = exp(S*scale - ln64): max ~66 << 240

# exp engine rotation per [P,2,NC_] unit: A=ACT exp, D=DVE trick+DVE cast,
# G=DVE trick+GpSimd cast
EXP_PAT = "AAAG" "ADAG" "AAAG" "ADAA"

_CACHED = {}


def build(taps=()):
    nc = bacc.Bacc("TRN2", debug=False)

    x_d = nc.dram_tensor("x", [N, C], F32, kind="ExternalInput")
    wqkv_d = nc.dram_tensor("w_qkv_e", [C, 3 * C], F8, kind="ExternalInput")
    wproj_d = nc.dram_tensor("w_proj_p", [H * P, C], F8, kind="ExternalInput")
    wfc1_d = nc.dram_tensor("w_fc1_e", [C, DFF], BF16, kind="ExternalInput")
    wfc2_d = nc.dram_tensor("w_fc2", [DFF, C], BF16, kind="ExternalInput")
    qkb_d = nc.dram_tensor("qk_bias", [P, 2 * H], F32, kind="ExternalInput")
    bf1_d = nc.dram_tensor("b_fc1_e", [DFF], F32, kind="ExternalInput")
    bf2_d = nc.dram_tensor("b_fc2", [C], F32, kind="ExternalInput")
    y_d = nc.dram_tensor("y", [N, C], F32, kind="ExternalOutput")

    tap_d = {}
    for name, shape, dt in [
        ("h_fm", [C, N], F8),
        ("q_fm", [H * P, N], BF16),
        ("k_fm", [H * P, N], BF16),
        ("v_ext", [N, H * VWE], F8),
        ("o_fm", [H * P, N], F8),
        ("x1", [N, C], F32),
        ("h2_fm", [C, N], BF16),
    ]:
        if name in taps:
            tap_d[name] = nc.dram_tensor(
                "tap_" + name, shape, dt, kind="ExternalOutput"
            )

    def bcast_row(dram_t, width):
        return bass.AP(tensor=dram_t, offset=0, ap=[[0, P], [1, width]])

    with TileContext(nc) as tc:
        consts = tc.alloc_tile_pool(name="consts", bufs=1, side="left")

        # ---------- constants ----------
        ident = consts.tile([P, P], BF16)
        make_identity(nc, ident)
        eps_t = consts.tile([P, 1], F32)
        nc.vector.memset(eps_t, EPS)
        qkb = consts.tile([P, 2 * H], F32)
        nc.gpsimd.dma_start(qkb[:], qkb_d[:, :])
        ebias = consts.tile([P, 1], F32)
        nc.vector.memset(ebias, -LNB)
        bf1c = consts.tile([P, FFT], F32)
        nc.gpsimd.dma_start(bf1c[:], bf1_d.rearrange("(t p) -> p t", p=P))

        # ---------- persistent tensors / early weight loads ----------
        xpool = tc.alloc_tile_pool(name="xpool", bufs=1, side="left")
        x1pool = tc.alloc_tile_pool(name="x1pool", bufs=1, side="right")
        wfc1p = tc.alloc_tile_pool(name="wfc1p", bufs=1, side="right")
        wprojp = tc.alloc_tile_pool(name="wprojp", bufs=1, side="left")
        wqkvp = tc.alloc_tile_pool(name="wqkvp", bufs=1, side="left")

        x_tok = xpool.tile([P, NT, C], F32)
        for nt in range(NT):
            nc.sync.dma_start(x_tok[:, nt, :], x_d[nt * P:(nt + 1) * P, :])

        wqkv = wqkvp.tile([P, KT, 3 * C], F8)
        for c0 in range(0, 3 * C, 1152):
            nc.sync.dma_start(
                wqkv[:, :, c0:c0 + 1152],
                wqkv_d.rearrange("(kt p) o -> p kt o", p=P)[:, :, c0:c0 + 1152],
            )
        wproj = wprojp.tile([P, H, C], F8)
        nc.gpsimd.dma_start(
            wproj[:], wproj_d.rearrange("(hb p) c -> p hb c", p=P)
        )
        wfc1 = wfc1p.tile([P, KT, DFF], BF16)
        nc.sync.dma_start(wfc1[:], wfc1_d.rearrange("(kt p) f -> p kt f", p=P))
        x1_tok = x1pool.tile([P, NT, C], F32)

        # ---------- helpers ----------
        def layernorm_tile(x_ap, h_tile, stats):
            """h_tile[:] = (x_ap - mean) * rstd, cast bf16."""
            st = stats.tile([P, 3, nc.vector.BN_STATS_DIM], F32, tag="bnst")
            for i in range(3):
                nc.vector.bn_stats(
                    out=st[:, i, :], in_=x_ap[:, i * 256:(i + 1) * 256]
                )
            mv = stats.tile([P, nc.vector.BN_AGGR_DIM], F32, tag="bnmv")
            nc.vector.bn_aggr(out=mv[:], in_=st[:])
            rstd = stats.tile([P, 1], F32, tag="rstd")
            nc.scalar.activation(
                out=rstd[:], in_=mv[:, 1:2],
                func=mybir.ActivationFunctionType.Sqrt,
                bias=eps_t[:], scale=1.0,
            )
            nc.vector.reciprocal(out=rstd[:], in_=rstd[:])
            nmr = stats.tile([P, 1], F32, tag="nmr")
            nc.vector.tensor_mul(nmr[:], mv[:, 0:1], rstd[:])
            nc.vector.tensor_scalar_mul(nmr[:], nmr[:], -1.0)
            nc.scalar.activation(
                out=h_tile[:], in_=x_ap[:],
                func=mybir.ActivationFunctionType.Identity,
                bias=nmr[:], scale=rstd[:],
            )

        def transpose_into(h_tile, dst_fm, nt, pst):
            """dst_fm[:, kt, nt*P:(nt+1)*P] = h_tile[P, C].T via PE."""
            for kt in range(KT):
                tp = pst.tile([P, P], BF16, tag="tp")
                nc.tensor.transpose(
                    tp[:], h_tile[:, kt * P:(kt + 1) * P], ident[:]
                )
                nc.vector.tensor_copy(
                    dst_fm[:, kt, nt * P:(nt + 1) * P], tp[:]
                )

        # ---------- phase 1: LN1 + transpose -> h_fm fp8 ----------
        hfmp = tc.alloc_tile_pool(name="hfmp", bufs=1, side="left")
        opool = tc.alloc_tile_pool(name="opool", bufs=1, side="left")
        lnt1 = tc.alloc_tile_pool(name="lnt1", bufs=3, side="left")
        stats1 = tc.alloc_tile_pool(name="stats1", bufs=4, side="left")
        pst = tc.alloc_tile_pool(name="pst", bufs=2, space="PSUM")

        h_fm = hfmp.tile([P, KT, N], F8)
        for nt in range(NT):
            h_t = lnt1.tile([P, C], BF16, tag="h1")
            layernorm_tile(x_tok[:, nt, :], h_t, stats1)
            transpose_into(h_t, h_fm, nt, pst)

        if "h_fm" in tap_d:
            nc.sync.dma_start(
                tap_d["h_fm"].rearrange("(kt p) n -> p kt n", p=P), h_fm[:]
            )
        stats1.release()
        lnt1.release()
        pst.release()

        # ---------- phase 2: v-gen (fp8 DoubleRow) ----------
        qkvpool = tc.alloc_tile_pool(name="qkvpool", bufs=1, side="right")
        vps = tc.alloc_tile_pool(name="vps", bufs=3, space="PSUM")

        q_fm = qkvpool.tile([P, H, N], BF16)
        k_fm = qkvpool.tile([P, H, N], BF16)
        v_ext = qkvpool.tile([P, NT, H, VWE], F8)
        o_fm = opool.tile([P, H, N], F8)
        nc.gpsimd.memset(v_ext[:, :, :, VWE - 1], 0.0)
        nc.gpsimd.memset(v_ext[:, :, :, 0], 1.0)
        nc.gpsimd.memset(o_fm[VW:P, :, :], 0.0)  # 32-aligned; row 96 rewritten

        for half in range(2):  # 4 heads (384 cols) per psum
            c0 = 2 * C + half * 4 * DH
            for nt in range(NT):
                pv = vps.tile([P, 4 * DH], F32, tag="v")
                for t in range(KT // 2):
                    nc.tensor.matmul(
                        pv[:],
                        h_fm[:, 2 * t:2 * t + 2, nt * P:(nt + 1) * P],
                        wqkv[:, 2 * t:2 * t + 2, c0:c0 + 4 * DH],
                        start=(t == 0), stop=(t == KT // 2 - 1),
                        perf_mode=DR,
                    )
                nc.vector.tensor_copy(
                    v_ext[:, nt, half * 4:(half + 1) * 4, 1:VW + 1],
                    pv.rearrange("p (h d) -> p h d", d=DH),
                )
        vps.release()

        # ---------- phase 3: merged qk + attention pipeline ----------
        epool = tc.alloc_tile_pool(name="epool", bufs=2, side="right")
        rrow = tc.alloc_tile_pool(name="rrow", bufs=3, side="right")
        sps = tc.alloc_tile_pool(name="sps", bufs=2, space="PSUM")
        pvps = tc.alloc_tile_pool(name="pvps", bufs=2, space="PSUM")
        qkps = tc.alloc_tile_pool(name="qkps", bufs=2, space="PSUM")

        exp_unit = [0]

        def emit_qk(h):
            for which, dst in ((0, q_fm), (1, k_fm)):
                col0 = which * C + h * DH
                for nh in range(NH):
                    pq = qkps.tile([P, NC_], F32, tag="qk")
                    for t in range(KT // 2):
                        nc.tensor.matmul(
                            pq[:DH, :],
                            wqkv[:, 2 * t:2 * t + 2, col0:col0 + DH],
                            h_fm[:, 2 * t:2 * t + 2, nh * NC_:(nh + 1) * NC_],
                            start=(t == 0), stop=(t == KT // 2 - 1),
                            perf_mode=DR,
                        )
                    nc.vector.tensor_scalar_add(
                        dst[:DH, h, nh * NC_:(nh + 1) * NC_],
                        pq[:DH, :],
                        qkb[:DH, which * H + h:which * H + h + 1],
                    )

        def emit_scores(h, nh):
            """e_t[:, mt, :] = exp(S^T*scale - ln64) fp8, all 8 key blocks."""
            e_t = epool.tile([P, NT, NC_], F8, tag="E", name=f"e_{h}_{nh}")
            for mt2 in range(NT // 2):
                ps_s = sps.tile([P, 2, NC_], F32, tag="S",
                                name=f"s_{h}_{nh}_{mt2}")
                for sub in range(2):
                    nc.tensor.matmul(
                        ps_s[:, sub, :],
                        k_fm[:DH, h,
                             (2 * mt2 + sub) * P:(2 * mt2 + sub + 1) * P],
                        q_fm[:DH, h, nh * NC_:(nh + 1) * NC_],
                        start=True, stop=True,
                    )
                eng = EXP_PAT[exp_unit[0] % len(EXP_PAT)]
                exp_unit[0] += 1
                if eng == "A":
                    nc.scalar.activation(
                        out=e_t[:, 2 * mt2:2 * mt2 + 2, :], in_=ps_s[:],
                        func=mybir.ActivationFunctionType.Exp,
                        bias=ebias[:], scale=SCALE,
                    )
                else:
                    it = rrow.tile([P, 2, NC_], I32, tag="it",
                                   name=f"it_{h}_{nh}_{mt2}")
                    nc.vector.tensor_scalar(
                        it[:], ps_s[:],
                        SCALE * EXP_A, EXP_B - LNB * EXP_A,
                        mybir.AluOpType.mult, mybir.AluOpType.add,
                    )
                    cast_eng = nc.vector if eng == "D" else nc.gpsimd
                    cast_eng.tensor_copy(
                        e_t[:, 2 * mt2:2 * mt2 + 2, :], it[:].bitcast(F32)
                    )
            return e_t

        def emit_pv(h, nh, e_t):
            po = pvps.tile([P, NC_], F32, tag="PV", name=f"po_{h}_{nh}")
            for t in range(NT // 2):
                nc.tensor.matmul(
                    po[:VWE, :],
                    v_ext[:, 2 * t:2 * t + 2, h, :],
                    e_t[:, 2 * t:2 * t + 2, :],
                    start=(t == 0), stop=(t == NT // 2 - 1),
                    perf_mode=DR,
                )
            # normalization: o = po / colsum  (row 0 of po is the colsum)
            rs = rrow.tile([1, NC_], F32, tag="rs", name=f"rs_{h}_{nh}")
            nc.scalar.copy(rs[0:1, :], po[0:1, :])
            rb = rrow.tile([P, NC_], F32, tag="rb", name=f"rb_{h}_{nh}")
            nc.gpsimd.partition_broadcast(rb[:VW + 1, :], rs[0:1, :])
            nc.vector.reciprocal_approx_fast(
                out=rb[:VW + 1, :], in_=rb[:VW + 1, :]
            )
            nc.vector.tensor_mul(
                o_fm[0:VW + 1, h, nh * NC_:(nh + 1) * NC_],
                po[0:VW + 1, :], rb[0:VW + 1, :],
            )

        pending = {}
        for s in range(H + 2):
            if s < H:
                emit_qk(s)
            if 1 <= s <= H:
                h = s - 1
                pending[h] = [emit_scores(h, nh) for nh in range(NH)]
            if s >= 2:
                h = s - 2
                for nh in range(NH):
                    emit_pv(h, nh, pending[h][nh])
                del pending[h]

        qkps.release()
        pjps = tc.alloc_tile_pool(name="pjps", bufs=2, space="PSUM")

        def emit_proj(nt):
            """x1 = x + o @ w_proj (+ b_proj via o_fm row0 x wproj row0)."""
            for c0, cw in ((0, 512), (512, 256)):
                pj = pjps.tile([P, 512], F32, tag="PJ", name=f"pj_{nt}_{c0}")
                for t in range(H // 2):
                    nc.tensor.matmul(
                        pj[:, :cw],
                        o_fm[:, 2 * t:2 * t + 2, nt * P:(nt + 1) * P],
                        wproj[:, 2 * t:2 * t + 2, c0:c0 + cw],
                        start=(t == 0), stop=(t == H // 2 - 1),
                        perf_mode=DR,
                    )
                nc.vector.tensor_add(
                    x1_tok[:, nt, c0:c0 + cw],
                    pj[:, :cw], x_tok[:, nt, c0:c0 + cw],
                )

        for nt in range(NT):
            emit_proj(nt)

        if "q_fm" in tap_d:
            nc.sync.dma_start(
                tap_d["q_fm"].rearrange("(h p) n -> p h n", p=P), q_fm[:]
            )
        if "k_fm" in tap_d:
            nc.sync.dma_start(
                tap_d["k_fm"].rearrange("(h p) n -> p h n", p=P), k_fm[:]
            )
        if "v_ext" in tap_d:
            nc.sync.dma_start(
                tap_d["v_ext"].rearrange(
                    "(nt p) (h w) -> p nt h w", p=P, w=VWE
                ),
                v_ext[:],
            )
        if "o_fm" in tap_d:
            nc.sync.dma_start(
                tap_d["o_fm"].rearrange("(h p) n -> p h n", p=P), o_fm[:]
            )
        if "x1" in tap_d:
            nc.sync.dma_start(
                tap_d["x1"].rearrange("(nt p) c -> p nt c", p=P), x1_tok[:]
            )

        pjps.release()
        pvps.release()
        sps.release()
        rrow.release()
        epool.release()
        qkvpool.release()
        opool.release()
        hfmp.release()
        wqkvp.release()
        wprojp.release()
        xpool.release()

        # ---------- late weight load: wfc2 resident (space freed by attn) ----
        wfc2p = tc.alloc_tile_pool(name="wfc2p", bufs=1, side="right")
        wfc2r = wfc2p.tile([P, FFT, C], BF16)
        for f0 in range(0, FFT, 4):
            nc.sync.dma_start(
                wfc2r[:, f0:f0 + 4, :],
                wfc2_d.rearrange("(ff p) c -> p ff c", p=P)[:, f0:f0 + 4, :],
            )

        # ---------- phase 4+5: LN2 interleaved with MLP (bf16) ----------
        h2p = tc.alloc_tile_pool(name="h2p", bufs=1, side="left")
        lnt2 = tc.alloc_tile_pool(name="lnt2", bufs=3, side="left")
        stats2 = tc.alloc_tile_pool(name="stats2", bufs=4, side="left")
        mlpc = tc.alloc_tile_pool(name="mlpc", bufs=1, side="left")
        gpool = tc.alloc_tile_pool(name="gpool", bufs=3, side="left")
        outs = tc.alloc_tile_pool(name="outs", bufs=2, side="left")
        pst2 = tc.alloc_tile_pool(name="pst2", bufs=2, space="PSUM")
        gps = tc.alloc_tile_pool(name="gps", bufs=2, space="PSUM")
        x2ps = tc.alloc_tile_pool(name="x2ps", bufs=2, space="PSUM")

        bf2b = mlpc.tile([P, C], F32)
        nc.gpsimd.dma_start(bf2b[:], bcast_row(bf2_d, C))
        h2_fm = h2p.tile([P, KT, N], BF16)

        QW = 256  # token quarter width
        for q in range(4):
            for nt in (2 * q, 2 * q + 1):
                h_t = lnt2.tile([P, C], BF16, tag="h2")
                layernorm_tile(x1_tok[:, nt, :], h_t, stats2)
                transpose_into(h_t, h2_fm, nt, pst2)
            pa = [x2ps.tile([P, 512], F32, tag="x2a", name=f"pa{q}_{jj}")
                  for jj in range(2)]
            pb = [x2ps.tile([P, 256], F32, tag="x2b", name=f"pb{q}_{jj}")
                  for jj in range(2)]
            for ff in range(FFT):
                pg = gps.tile([P, QW], F32, tag="G")
                for kt in range(KT):
                    nc.tensor.matmul(
                        pg[:],
                        wfc1[:, kt, ff * P:(ff + 1) * P],
                        h2_fm[:, kt, q * QW:(q + 1) * QW],
                        start=(kt == 0), stop=(kt == KT - 1),
                    )
                g_t = gpool.tile([P, QW], BF16, tag="g")
                nc.scalar.activation(
                    out=g_t[:], in_=pg[:],
                    func=mybir.ActivationFunctionType.Gelu,
                    bias=bf1c[:, ff:ff + 1], scale=1.0,
                )
                for j in range(2):
                    nc.tensor.matmul(
                        pa[j][:],
                        g_t[:, j * P:(j + 1) * P],
                        wfc2r[:, ff, 0:512],
                        start=(ff == 0), stop=(ff == FFT - 1),
                    )
                    nc.tensor.matmul(
                        pb[j][:],
                        g_t[:, j * P:(j + 1) * P],
                        wfc2r[:, ff, 512:768],
                        start=(ff == 0), stop=(ff == FFT - 1),
                    )
            for j in range(2):
                nt = 2 * q + j
                o_t = outs.tile([P, C], F32, tag="y")
                nc.vector.tensor_add(
                    o_t[:, 0:512], pa[j][:], x1_tok[:, nt, 0:512]
                )
                nc.vector.tensor_add(
                    o_t[:, 512:768], pb[j][:], x1_tok[:, nt, 512:768]
                )
                nc.vector.tensor_add(o_t[:], o_t[:], bf2b[:])
                nc.sync.dma_start(y_d[nt * P:(nt + 1) * P, :], o_t[:])

        if "h2_fm" in tap_d:
            nc.sync.dma_start(
                tap_d["h2_fm"].rearrange("(kt p) n -> p kt n", p=P), h2_fm[:]
            )

        x2ps.release()
        gps.release()
        pst2.release()
        outs.release()
        gpool.release()
        mlpc.release()
        stats2.release()
        lnt2.release()
        h2p.release()
        wfc2p.release()
        wfc1p.release()
        x1pool.release()
        consts.release()

    nc.compile()
    return nc


def _prep_inputs(inputs):
    """Host-side prep (exact refactoring of LN gains/biases into weights)."""
    f = lambda k: np.asarray(inputs[k], dtype=np.float32)
    x = f("x")
    w_qkv, w_proj, w_fc1, w_fc2 = f("w_qkv"), f("w_proj"), f("w_fc1"), f("w_fc2")
    ln1_g, ln1_b, ln2_g, ln2_b = f("ln1_g"), f("ln1_b"), f("ln2_g"), f("ln2_b")
    b_proj, b_fc1, b_fc2 = f("b_proj"), f("b_fc1"), f("b_fc2")

    bf = ml_dtypes.bfloat16
    f8 = ml_dtypes.float8_e4m3
    w_qkv_e = ln1_g[:, None] * w_qkv
    qkv_bias = ln1_b @ w_qkv  # [2304]
    qk_bias = np.zeros((P, 2 * H), dtype=np.float32)
    for which in range(2):
        for h in range(H):
            qk_bias[0:DH, which * H + h] = qkv_bias[
                which * C + h * DH: which * C + (h + 1) * DH
            ]
    vb = qkv_bias[2 * C: 3 * C]  # v bias passes through softmax additively
    b_proj_e = b_proj + vb @ w_proj
    # head-aligned w_proj rows: block h rows 1..97 (row 0 of block 0 carries
    # b_proj_e, multiplied by the all-ones row 0 of o_fm)
    w_proj_p = np.zeros((H * P, C), dtype=np.float32)
    for h in range(H):
        w_proj_p[h * P + 1: h * P + 1 + DH, :] = w_proj[h * DH:(h + 1) * DH, :]
    w_proj_p[0, :] = b_proj_e
    w_fc1_e = ln2_g[:, None] * w_fc1
    b_fc1_e = b_fc1 + ln2_b @ w_fc1

    common = {
        "w_qkv_e": np.ascontiguousarray(w_qkv_e.astype(f8)),
        "w_proj_p": np.ascontiguousarray(w_proj_p.astype(f8)),
        "w_fc1_e": np.ascontiguousarray(w_fc1_e.astype(bf)),
        "w_fc2": np.ascontiguousarray(w_fc2.astype(bf)),
        "qk_bias": qk_bias,
        "b_fc1_e": b_fc1_e,
        "b_fc2": b_fc2,
    }
    return [dict(common, x=np.ascontiguousarray(x[i])) for i in range(8)]


def kernel(**inputs):
    if "nc" not in _CACHED:
        _CACHED["nc"] = build()
    nc = _CACHED["nc"]
    in_maps = _prep_inputs(inputs)
    res = run_bass_kernel_spmd(nc, in_maps, core_ids=list(range(8)))
    out = np.stack([res.results[i]["y"] for i in range(8)], axis=0)
    return out.astype(np.float32)


# revision 14
# speedup vs baseline: 1.2029x; 1.0756x over previous
"""Transformer block (pre-norm attn + MLP) on 8 NeuronCores, data-parallel over batch.

Full inputs in, full outputs out. Each core processes one batch element
x[i] : [1024, 768] through an identical Bass/Tile kernel.

Host-side exact refactoring (as baseline):
  - LN gains fold into following matmul weights: diag(g) @ W.
  - LN biases fold into q/k per-column bias, b_proj_eff, b_fc1_eff.
  - w_proj rows head-aligned: block h rows 1..97; block-0 row 0 carries
    b_proj_eff (o_fm row 0 is exactly 1.0 after normalization).
  - w_qkv / w_proj cast to fp8 e4m3; w_fc1 / w_fc2 to bf16.

v2.1 on-chip structure:
  - qkv / v-gen / PV / proj matmuls run fp8 DoubleRow (2 contraction
    tiles fused per instruction, 2x rate on HW). Scores stay bf16
    (K=96 gains nothing from DoubleRow).
  - merged qkv+attention software pipeline: v-gen first, then per-head
    steps qk(h) | S(h-1)+exp | PV(h-2)+norm so the exp stream overlaps
    the qkv matmuls.
  - exp split across engines (ACT direct exp; Schraudolph int32
    bit-trick on DVE + fp8 cast on DVE/GpSimd) with a -ln64 bias so
    e^S/64 <= ~66 stays well under the fp8 e4m3 max finite 240.
  - softmax normalization: ACT copies the colsum row out of PSUM,
    GpSimd broadcasts, DVE fast-reciprocal + multiply -> o_fm fp8.
  - wfc2 resident in SBUF; LN2 interleaved with the MLP per token
    quarter so fc matmuls start as soon as their h2 quarter is ready.
"""
import numpy as np
import ml_dtypes

import concourse.bass as bass
from concourse import bacc, mybir
from concourse.bass_utils import run_bass_kernel_spmd
from concourse.masks import make_identity
from concourse.tile import TileContext

P = 128
N = 1024          # tokens per core (batch element)
C = 768           # model dim
H = 8             # heads
DH = C // H       # 96
DFF = 4 * C       # 3072
NT = N // P       # 8 token tiles
KT = C // P       # 6 feature tiles
FFT = DFF // P    # 24 ff tiles
NH = 2            # halves of the token axis for attention
NC_ = N // NH     # 512
EPS = 1e-5
SCALE = DH ** -0.5
VW = DH           # per-head v width
VWE = VW + 2      # slot: [ones | v(96) | zero pad] (dual-fp8 ldweights needs even width)

F32 = mybir.dt.float32
BF16 = mybir.dt.bfloat16
F8 = mybir.dt.float8e4
I32 = mybir.dt.int32
DR = mybir.MatmulPerfMode.DoubleRow

# exp(x) ~= bitcast_f32(int32(x * 2^23*log2e + (127*2^23 - C0)))
LOG2E = 1.4426950408889634
SH_C = 486411.0           # min-RMS Schraudolph correction
EXP_A = (1 << 23) * LOG2E
EXP_B = 127 * (1 << 23) - SH_C
LNB = float(np.log(32.0))  # E = exp(S*scale - ln32): ACT max ~132 < 240; trick max byte ~112 < 120

# exp engine rotation per [P,2,NC_] unit: A=ACT exp (exact, incl subnormal
# E values), D=DVE single-pass Schraudolph-to-fp8-bytes (zeroes the subnormal
# tail; ~6% of softmax mass on those tiles, randomly-signed effect on o)
EXP_PAT = "AADADADA"
U8 = mybir.dt.uint8
E8A = 8.0 * LOG2E          # byte = 8*log2(e^x/32) + 56 = x*E8A + E8B
E8B = 16.0 - 0.344         # Schraudolph min-RMS shift in byte units

_CACHED = {}


def build(taps=()):
    nc = bacc.Bacc("TRN2", debug=False)

    x_d = nc.dram_tensor("x", [N, C], F32, kind="ExternalInput")
    wqkv_d = nc.dram_tensor("w_qkv_e", [C, 3 * C], F8, kind="ExternalInput")
    wproj_d = nc.dram_tensor("w_proj_p", [H * P, C], F8, kind="ExternalInput")
    wfc1_d = nc.dram_tensor("w_fc1_e", [C, DFF], BF16, kind="ExternalInput")
    wfc2_d = nc.dram_tensor("w_fc2", [DFF, C], BF16, kind="ExternalInput")
    qkb_d = nc.dram_tensor("qk_bias", [P, 2 * H], F32, kind="ExternalInput")
    bf1_d = nc.dram_tensor("b_fc1_e", [DFF], F32, kind="ExternalInput")
    bf2_d = nc.dram_tensor("b_fc2", [C], F32, kind="ExternalInput")
    y_d = nc.dram_tensor("y", [N, C], F32, kind="ExternalOutput")

    tap_d = {}
    for name, shape, dt in [
        ("h_fm", [C, N], F8),
        ("q_fm", [H * P, N], BF16),
        ("k_fm", [H * P, N], BF16),
        ("v_ext", [N, H * VWE], F8),
        ("o_fm", [H * P, N], F8),
        ("x1", [N, C], F32),
        ("h2_fm", [C, N], BF16),
    ]:
        if name in taps:
            tap_d[name] = nc.dram_tensor(
                "tap_" + name, shape, dt, kind="ExternalOutput"
            )

    def bcast_row(dram_t, width):
        return bass.AP(tensor=dram_t, offset=0, ap=[[0, P], [1, width]])

    with TileContext(nc) as tc:
        consts = tc.alloc_tile_pool(name="consts", bufs=1, side="left")

        # ---------- constants ----------
        ident = consts.tile([P, P], BF16)
        make_identity(nc, ident)
        eps_t = consts.tile([P, 1], F32)
        nc.vector.memset(eps_t, EPS)
        qkb = consts.tile([P, 2 * H], F32)
        nc.gpsimd.dma_start(qkb[:], qkb_d[:, :])
        ebias = consts.tile([P, 1], F32)
        nc.vector.memset(ebias, -LNB)
        bf1c = consts.tile([P, FFT], F32)
        nc.gpsimd.dma_start(bf1c[:], bf1_d.rearrange("(t p) -> p t", p=P))

        # ---------- persistent tensors / early weight loads ----------
        xpool = tc.alloc_tile_pool(name="xpool", bufs=1, side="left")
        x1pool = tc.alloc_tile_pool(name="x1pool", bufs=1, side="right")
        wfc1p = tc.alloc_tile_pool(name="wfc1p", bufs=1, side="right")
        wprojp = tc.alloc_tile_pool(name="wprojp", bufs=1, side="left")
        wqkvp = tc.alloc_tile_pool(name="wqkvp", bufs=1, side="left")

        x_tok = xpool.tile([P, NT, C], F32)
        for nt in range(NT):
            nc.sync.dma_start(x_tok[:, nt, :], x_d[nt * P:(nt + 1) * P, :])

        wqkv = wqkvp.tile([P, KT, 3 * C], F8)
        for c0 in range(0, 3 * C, 1152):
            nc.sync.dma_start(
                wqkv[:, :, c0:c0 + 1152],
                wqkv_d.rearrange("(kt p) o -> p kt o", p=P)[:, :, c0:c0 + 1152],
            )
        wproj = wprojp.tile([P, H, C], F8)
        nc.gpsimd.dma_start(
            wproj[:], wproj_d.rearrange("(hb p) c -> p hb c", p=P)
        )
        wfc1 = wfc1p.tile([P, KT, DFF], BF16)
        nc.sync.dma_start(wfc1[:], wfc1_d.rearrange("(kt p) f -> p kt f", p=P))
        x1_tok = x1pool.tile([P, NT, C], F32)

        # ---------- helpers ----------
        def layernorm_tile(x_ap, h_tile, stats):
            """h_tile[:] = (x_ap - mean) * rstd, cast bf16."""
            st = stats.tile([P, 3, nc.vector.BN_STATS_DIM], F32, tag="bnst")
            for i in range(3):
                nc.vector.bn_stats(
                    out=st[:, i, :], in_=x_ap[:, i * 256:(i + 1) * 256]
                )
            mv = stats.tile([P, nc.vector.BN_AGGR_DIM], F32, tag="bnmv")
            nc.vector.bn_aggr(out=mv[:], in_=st[:])
            rstd = stats.tile([P, 1], F32, tag="rstd")
            nc.scalar.activation(
                out=rstd[:], in_=mv[:, 1:2],
                func=mybir.ActivationFunctionType.Sqrt,
                bias=eps_t[:], scale=1.0,
            )
            nc.vector.reciprocal(out=rstd[:], in_=rstd[:])
            nmr = stats.tile([P, 1], F32, tag="nmr")
            nc.vector.tensor_mul(nmr[:], mv[:, 0:1], rstd[:])
            nc.vector.tensor_scalar_mul(nmr[:], nmr[:], -1.0)
            nc.scalar.activation(
                out=h_tile[:], in_=x_ap[:],
                func=mybir.ActivationFunctionType.Identity,
                bias=nmr[:], scale=rstd[:],
            )

        def transpose_into(h_tile, dst_fm, nt, pst):
            """dst_fm[:, kt, nt*P:(nt+1)*P] = h_tile[P, C].T via PE."""
            for kt in range(KT):
                tp = pst.tile([P, P], BF16, tag="tp")
                nc.tensor.transpose(
                    tp[:], h_tile[:, kt * P:(kt + 1) * P], ident[:]
                )
                nc.vector.tensor_copy(
                    dst_fm[:, kt, nt * P:(nt + 1) * P], tp[:]
                )

        # ---------- phase 1: LN1 + transpose -> h_fm fp8 ----------
        hfmp = tc.alloc_tile_pool(name="hfmp", bufs=1, side="left")
        opool = tc.alloc_tile_pool(name="opool", bufs=1, side="left")
        lnt1 = tc.alloc_tile_pool(name="lnt1", bufs=3, side="left")
        stats1 = tc.alloc_tile_pool(name="stats1", bufs=4, side="left")
        pst = tc.alloc_tile_pool(name="pst", bufs=2, space="PSUM")

        h_fm = hfmp.tile([P, KT, N], F8)
        for nt in range(NT):
            h_t = lnt1.tile([P, C], BF16, tag="h1")
            layernorm_tile(x_tok[:, nt, :], h_t, stats1)
            transpose_into(h_t, h_fm, nt, pst)

        if "h_fm" in tap_d:
            nc.sync.dma_start(
                tap_d["h_fm"].rearrange("(kt p) n -> p kt n", p=P), h_fm[:]
            )
        stats1.release()
        lnt1.release()
        pst.release()

        # ---------- phase 2: v-gen (fp8 DoubleRow) ----------
        qkvpool = tc.alloc_tile_pool(name="qkvpool", bufs=1, side="right")
        vps = tc.alloc_tile_pool(name="vps", bufs=3, space="PSUM")

        q_fm = qkvpool.tile([P, H, N], BF16)
        k_fm = qkvpool.tile([P, H, N], BF16)
        v_ext = qkvpool.tile([P, NT, H, VWE], F8)
        o_fm = opool.tile([P, H, N], F8)
        nc.gpsimd.memset(v_ext[:, :, :, VWE - 1], 0.0)
        nc.gpsimd.memset(v_ext[:, :, :, 0], 1.0)
        nc.gpsimd.memset(o_fm[VW:P, :, :], 0.0)  # 32-aligned; row 96 rewritten

        for half in range(2):  # 4 heads (384 cols) per psum
            c0 = 2 * C + half * 4 * DH
            for nt in range(NT):
                pv = vps.tile([P, 4 * DH], F32, tag="v")
                for t in range(KT // 2):
                    nc.tensor.matmul(
                        pv[:],
                        h_fm[:, 2 * t:2 * t + 2, nt * P:(nt + 1) * P],
                        wqkv[:, 2 * t:2 * t + 2, c0:c0 + 4 * DH],
                        start=(t == 0), stop=(t == KT // 2 - 1),
                        perf_mode=DR,
                    )
                nc.vector.tensor_copy(
                    v_ext[:, nt, half * 4:(half + 1) * 4, 1:VW + 1],
                    pv.rearrange("p (h d) -> p h d", d=DH),
                )
        vps.release()

        # ---------- phase 3: merged qk + attention pipeline ----------
        epool = tc.alloc_tile_pool(name="epool", bufs=4, side="right")
        rrow = tc.alloc_tile_pool(name="rrow", bufs=3, side="right")
        sps = tc.alloc_tile_pool(name="sps", bufs=2, space="PSUM")
        pvps = tc.alloc_tile_pool(name="pvps", bufs=2, space="PSUM")
        qkps = tc.alloc_tile_pool(name="qkps", bufs=2, space="PSUM")

        exp_unit = [0]

        def make_qk_tiles(h):
            tiles = []
            for which, dst in ((0, q_fm), (1, k_fm)):
                for nh in range(NH):
                    def emit(which=which, dst=dst, nh=nh, h=h):
                        col0 = which * C + h * DH
                        pq = qkps.tile([P, NC_], F32, tag="qk")
                        for t in range(KT // 2):
                            nc.tensor.matmul(
                                pq[:DH, :],
                                wqkv[:, 2 * t:2 * t + 2, col0:col0 + DH],
                                h_fm[:, 2 * t:2 * t + 2,
                                     nh * NC_:(nh + 1) * NC_],
                                start=(t == 0), stop=(t == KT // 2 - 1),
                                perf_mode=DR,
                            )
                        nc.vector.tensor_scalar_add(
                            dst[:DH, h, nh * NC_:(nh + 1) * NC_],
                            pq[:DH, :],
                            qkb[:DH, which * H + h:which * H + h + 1],
                        )
                    tiles.append(emit)
            return tiles

        def make_s_tiles(h, e_pair):
            tiles = []
            for nh in range(NH):
                for mt2 in range(NT // 2):
                    def emit(h=h, nh=nh, mt2=mt2, e_t=e_pair[nh]):
                        ps_s = sps.tile([P, 2, NC_], F32, tag="S",
                                        name=f"s_{h}_{nh}_{mt2}")
                        for sub in range(2):
                            nc.tensor.matmul(
                                ps_s[:, sub, :],
                                k_fm[:DH, h, (2 * mt2 + sub) * P:
                                     (2 * mt2 + sub + 1) * P],
                                q_fm[:DH, h, nh * NC_:(nh + 1) * NC_],
                                start=True, stop=True,
                            )
                        eng = EXP_PAT[exp_unit[0] % len(EXP_PAT)]
                        exp_unit[0] += 1
                        if eng == "A":
                            nc.scalar.activation(
                                out=e_t[:, 2 * mt2:2 * mt2 + 2, :],
                                in_=ps_s[:],
                                func=mybir.ActivationFunctionType.Exp,
                                bias=ebias[:], scale=SCALE,
                            )
                        else:
                            nc.vector.tensor_scalar(
                                e_t[:, 2 * mt2:2 * mt2 + 2, :].bitcast(U8),
                                ps_s[:],
                                SCALE * E8A, E8B,
                                mybir.AluOpType.mult, mybir.AluOpType.add,
                            )
                    tiles.append(emit)
            return tiles

        def emit_pv(h, nh, e_t):
            po = pvps.tile([P, NC_], F32, tag="PV", name=f"po_{h}_{nh}")
            for t in range(NT // 2):
                nc.tensor.matmul(
                    po[:VWE, :],
                    v_ext[:, 2 * t:2 * t + 2, h, :],
                    e_t[:, 2 * t:2 * t + 2, :],
                    start=(t == 0), stop=(t == NT // 2 - 1),
                    perf_mode=DR,
                )
            # normalization: o = po / colsum  (row 0 of po is the colsum)
            rs = rrow.tile([1, NC_], F32, tag="rs", name=f"rs_{h}_{nh}")
            nc.scalar.copy(rs[0:1, :], po[0:1, :])
            rb = rrow.tile([P, NC_], F32, tag="rb", name=f"rb_{h}_{nh}")
            nc.gpsimd.partition_broadcast(rb[:VW + 1, :], rs[0:1, :])
            nc.vector.reciprocal_approx_fast(
                out=rb[:VW + 1, :], in_=rb[:VW + 1, :]
            )
            nc.vector.tensor_mul(
                o_fm[0:VW + 1, h, nh * NC_:(nh + 1) * NC_],
                po[0:VW + 1, :], rb[0:VW + 1, :],
            )

        pending = {}
        for s in range(H + 2):
            qk_tiles = make_qk_tiles(s) if s < H else []
            s_tiles = []
            if 1 <= s <= H:
                h = s - 1
                e_pair = [
                    epool.tile([P, NT, NC_], F8, tag="E", name=f"e_{h}_{nh}")
                    for nh in range(NH)
                ]
                pending[h] = e_pair
                s_tiles = make_s_tiles(h, e_pair)
            pv_jobs = []
            if s >= 2:
                h = s - 2
                pv_jobs = [(h, nh, pending[h][nh]) for nh in range(NH)]

            # interleave: qk0 s0 s1 qk1 PV0 s2 s3 qk2 PV1 s4 s5 qk3 s6 s7
            slots = []
            qi, si = 0, 0
            for r in range(4):
                if qi < len(qk_tiles):
                    slots.append(qk_tiles[qi]); qi += 1
                if r == 1 and pv_jobs:
                    slots.append(lambda j=pv_jobs[0]: emit_pv(*j))
                if r == 2 and len(pv_jobs) > 1:
                    slots.append(lambda j=pv_jobs[1]: emit_pv(*j))
                for _ in range(2):
                    if si < len(s_tiles):
                        slots.append(s_tiles[si]); si += 1
            while si < len(s_tiles):
                slots.append(s_tiles[si]); si += 1
            if not qk_tiles and not s_tiles:
                for j in pv_jobs:
                    slots.append(lambda j=j: emit_pv(*j))
            elif not s_tiles:
                pass
            for fn in slots:
                fn()
            if s >= 2:
                del pending[s - 2]

        qkps.release()
        pjps = tc.alloc_tile_pool(name="pjps", bufs=2, space="PSUM")

        def emit_proj(nt):
            """x1 = x + o @ w_proj (+ b_proj via o_fm row0 x wproj row0)."""
            for c0, cw in ((0, 512), (512, 256)):
                pj = pjps.tile([P, 512], F32, tag="PJ", name=f"pj_{nt}_{c0}")
                for t in range(H // 2):
                    nc.tensor.matmul(
                        pj[:, :cw],
                        o_fm[:, 2 * t:2 * t + 2, nt * P:(nt + 1) * P],
                        wproj[:, 2 * t:2 * t + 2, c0:c0 + cw],
                        start=(t == 0), stop=(t == H // 2 - 1),
                        perf_mode=DR,
                    )
                nc.vector.tensor_add(
                    x1_tok[:, nt, c0:c0 + cw],
                    pj[:, :cw], x_tok[:, nt, c0:c0 + cw],
                )

        for nt in range(NT):
            emit_proj(nt)

        if "q_fm" in tap_d:
            nc.sync.dma_start(
                tap_d["q_fm"].rearrange("(h p) n -> p h n", p=P), q_fm[:]
            )
        if "k_fm" in tap_d:
            nc.sync.dma_start(
                tap_d["k_fm"].rearrange("(h p) n -> p h n", p=P), k_fm[:]
            )
        if "v_ext" in tap_d:
            nc.sync.dma_start(
                tap_d["v_ext"].rearrange(
                    "(nt p) (h w) -> p nt h w", p=P, w=VWE
                ),
                v_ext[:],
            )
        if "o_fm" in tap_d:
            nc.sync.dma_start(
                tap_d["o_fm"].rearrange("(h p) n -> p h n", p=P), o_fm[:]
            )
        if "x1" in tap_d:
            nc.sync.dma_start(
                tap_d["x1"].rearrange("(nt p) c -> p nt c", p=P), x1_tok[:]
            )

        pjps.release()
        pvps.release()
        sps.release()
        rrow.release()
        epool.release()
        qkvpool.release()
        opool.release()
        hfmp.release()
        wqkvp.release()
        wprojp.release()
        xpool.release()

        # ---------- late weight load: wfc2 resident (space freed by attn) ----
        wfc2p = tc.alloc_tile_pool(name="wfc2p", bufs=1, side="right")
        wfc2r = wfc2p.tile([P, FFT, C], BF16)
        for f0 in range(0, FFT, 4):
            nc.sync.dma_start(
                wfc2r[:, f0:f0 + 4, :],
                wfc2_d.rearrange("(ff p) c -> p ff c", p=P)[:, f0:f0 + 4, :],
            )

        # ---------- phase 4+5: LN2 interleaved with MLP (bf16) ----------
        h2p = tc.alloc_tile_pool(name="h2p", bufs=1, side="left")
        lnt2 = tc.alloc_tile_pool(name="lnt2", bufs=3, side="left")
        stats2 = tc.alloc_tile_pool(name="stats2", bufs=4, side="left")
        mlpc = tc.alloc_tile_pool(name="mlpc", bufs=1, side="left")
        gpool = tc.alloc_tile_pool(name="gpool", bufs=3, side="left")
        outs = tc.alloc_tile_pool(name="outs", bufs=2, side="left")
        pst2 = tc.alloc_tile_pool(name="pst2", bufs=2, space="PSUM")
        gps = tc.alloc_tile_pool(name="gps", bufs=2, space="PSUM")
        x2ps = tc.alloc_tile_pool(name="x2ps", bufs=2, space="PSUM")

        bf2b = mlpc.tile([P, C], F32)
        nc.gpsimd.dma_start(bf2b[:], bcast_row(bf2_d, C))
        h2_fm = h2p.tile([P, KT, N], BF16)

        QW = 256  # token quarter width
        for q in range(4):
            for nt in (2 * q, 2 * q + 1):
                h_t = lnt2.tile([P, C], BF16, tag="h2")
                layernorm_tile(x1_tok[:, nt, :], h_t, stats2)
                transpose_into(h_t, h2_fm, nt, pst2)
            pa = [x2ps.tile([P, 512], F32, tag="x2a", name=f"pa{q}_{jj}")
                  for jj in range(2)]
            pb = [x2ps.tile([P, 256], F32, tag="x2b", name=f"pb{q}_{jj}")
                  for jj in range(2)]
            for ff in range(FFT):
                pg = gps.tile([P, QW], F32, tag="G")
                for kt in range(KT):
                    nc.tensor.matmul(
                        pg[:],
                        wfc1[:, kt, ff * P:(ff + 1) * P],
                        h2_fm[:, kt, q * QW:(q + 1) * QW],
                        start=(kt == 0), stop=(kt == KT - 1),
                    )
                g_t = gpool.tile([P, QW], BF16, tag="g")
                nc.scalar.activation(
                    out=g_t[:], in_=pg[:],
                    func=mybir.ActivationFunctionType.Gelu,
                    bias=bf1c[:, ff:ff + 1], scale=1.0,
                )
                for j in range(2):
                    nc.tensor.matmul(
                        pa[j][:],
                        g_t[:, j * P:(j + 1) * P],
                        wfc2r[:, ff, 0:512],
                        start=(ff == 0), stop=(ff == FFT - 1),
                    )
                    nc.tensor.matmul(
                        pb[j][:],
                        g_t[:, j * P:(j + 1) * P],
                        wfc2r[:, ff, 512:768],
                        start=(ff == 0), stop=(ff == FFT - 1),
                    )
            for j in range(2):
                nt = 2 * q + j
                o_t = outs.tile([P, C], F32, tag="y")
                nc.vector.tensor_add(
                    o_t[:, 0:512], pa[j][:], x1_tok[:, nt, 0:512]
                )
                nc.vector.tensor_add(
                    o_t[:, 512:768], pb[j][:], x1_tok[:, nt, 512:768]
                )
                nc.vector.tensor_add(o_t[:], o_t[:], bf2b[:])
                nc.sync.dma_start(y_d[nt * P:(nt + 1) * P, :], o_t[:])

        if "h2_fm" in tap_d:
            nc.sync.dma_start(
                tap_d["h2_fm"].rearrange("(kt p) n -> p kt n", p=P), h2_fm[:]
            )

        x2ps.release()
        gps.release()
        pst2.release()
        outs.release()
        gpool.release()
        mlpc.release()
        stats2.release()
        lnt2.release()
        h2p.release()
        wfc2p.release()
        wfc1p.release()
        x1pool.release()
        consts.release()

    nc.compile()
    return nc


def _prep_inputs(inputs):
    """Host-side prep (exact refactoring of LN gains/biases into weights)."""
    f = lambda k: np.asarray(inputs[k], dtype=np.float32)
    x = f("x")
    w_qkv, w_proj, w_fc1, w_fc2 = f("w_qkv"), f("w_proj"), f("w_fc1"), f("w_fc2")
    ln1_g, ln1_b, ln2_g, ln2_b = f("ln1_g"), f("ln1_b"), f("ln2_g"), f("ln2_b")
    b_proj, b_fc1, b_fc2 = f("b_proj"), f("b_fc1"), f("b_fc2")

    bf = ml_dtypes.bfloat16
    f8 = ml_dtypes.float8_e4m3
    w_qkv_e = ln1_g[:, None] * w_qkv
    qkv_bias = ln1_b @ w_qkv  # [2304]
    qk_bias = np.zeros((P, 2 * H), dtype=np.float32)
    for which in range(2):
        for h in range(H):
            qk_bias[0:DH, which * H + h] = qkv_bias[
                which * C + h * DH: which * C + (h + 1) * DH
            ]
    vb = qkv_bias[2 * C: 3 * C]  # v bias passes through softmax additively
    b_proj_e = b_proj + vb @ w_proj
    # head-aligned w_proj rows: block h rows 1..97 (row 0 of block 0 carries
    # b_proj_e, multiplied by the all-ones row 0 of o_fm)
    w_proj_p = np.zeros((H * P, C), dtype=np.float32)
    for h in range(H):
        w_proj_p[h * P + 1: h * P + 1 + DH, :] = w_proj[h * DH:(h + 1) * DH, :]
    w_proj_p[0, :] = b_proj_e
    w_fc1_e = ln2_g[:, None] * w_fc1
    b_fc1_e = b_fc1 + ln2_b @ w_fc1

    common = {
        "w_qkv_e": np.ascontiguousarray(w_qkv_e.astype(f8)),
        "w_proj_p": np.ascontiguousarray(w_proj_p.astype(f8)),
        "w_fc1_e": np.ascontiguousarray(w_fc1_e.astype(bf)),
        "w_fc2": np.ascontiguousarray(w_fc2.astype(bf)),
        "qk_bias": qk_bias,
        "b_fc1_e": b_fc1_e,
        "b_fc2": b_fc2,
    }
    return [dict(common, x=np.ascontiguousarray(x[i])) for i in range(8)]


def kernel(**inputs):
    if "nc" not in _CACHED:
        _CACHED["nc"] = build()
    nc = _CACHED["nc"]
    in_maps = _prep_inputs(inputs)
    res = run_bass_kernel_spmd(nc, in_maps, core_ids=list(range(8)))
    out = np.stack([res.results[i]["y"] for i in range(8)], axis=0)
    return out.astype(np.float32)
